# revision 16
# baseline (speedup 1.0000x reference)
"""Trainium2 Bass kernel for nn_BilinearFusion.

out[b] = sum_h [ x1_h(b)·W1_h + b1_h + x2_h(b)·W2_h + x2_h(b)^T W3_h x1_h(b) ]

Host-side staging: shard batch across 8 cores; cast x1/x2 to bf16 and lay
them out pre-transposed per head (xt[i, h, b] = x[b, h*128+i]) in batch-major
1MB chunks so the device only does contiguous DMA loads.

Device (per core, 2048 rows = 4 batches x 512):
  loop h over heads (W3_h^T stays stationary across batches):
    per batch:
      Yt[o, b]  = W3_h^T @ xt_h                  (PE, bf16 -> fp32 PSUM)
      prod[o,b] = Yt * x2t_h                      (DVE tensor_mul)
      res_b[1,b] += W1_h^T @ xt_h                 (PE M=1: t1)
      res_b[1,b] += W2_h^T @ x2t_h                (PE M=1: t2)
      res_b[1,b] += ones^T @ prod                 (PE M=1: t3)
  copy each res row to SBUF, DMA out.  Host adds sum(b1).
"""

import numpy as np
import ml_dtypes

import concourse.bass as bass
import concourse.tile as tile
from concourse import bacc, mybir
from concourse.bass_utils import run_bass_kernel_spmd

BF16 = ml_dtypes.bfloat16

B, D, HEAD, DIM = 16384, 1024, 8, 128
NCORES = 8
ROWS = B // NCORES          # 2048 rows per core
P = 128
BATCH = 512                 # rows per batch (moving free dim of matmuls)
NB = ROWS // BATCH          # 4 batches

_nc_cache = []


def build_nc():
    nc = bacc.Bacc(target_bir_lowering=False)
    f32 = mybir.dt.float32
    bf16 = mybir.dt.bfloat16

    x1t_d = nc.dram_tensor("x1t", [NB, P, HEAD, BATCH], bf16,
                           kind="ExternalInput")
    x2t_d = nc.dram_tensor("x2t", [NB, P, HEAD, BATCH], bf16,
                           kind="ExternalInput")
    w3t_d = nc.dram_tensor("w3t", [DIM, HEAD, DIM], bf16, kind="ExternalInput")
    w12_d = nc.dram_tensor("w12", [DIM, 2, HEAD], bf16, kind="ExternalInput")
    out_d = nc.dram_tensor("out", [NB, BATCH], f32, kind="ExternalOutput")

    with tile.TileContext(nc) as tc:
        with (
            tc.tile_pool(name="const", bufs=1) as const_pool,
            tc.tile_pool(name="xt", bufs=3) as xt_pool,
            tc.tile_pool(name="ysb", bufs=4) as ysb_pool,
            tc.tile_pool(name="prod", bufs=4) as prod_pool,
            tc.tile_pool(name="p1", bufs=10) as p1_pool,
            tc.tile_pool(name="res", bufs=2) as res_pool,
            tc.tile_pool(name="yps", bufs=4, space="PSUM") as yps_pool,
            tc.tile_pool(name="rps", bufs=2, space="PSUM") as rps_pool,
        ):
            w3l = const_pool.tile([DIM, HEAD, DIM], bf16)
            nc.sync.dma_start(out=w3l[:, 0:2, :], in_=w3t_d[:, 0:2, :])
            nc.sync.dma_start(out=w3l[:, 2:8, :], in_=w3t_d[:, 2:8, :])
            w12c = const_pool.tile([DIM, 2, HEAD], bf16)
            nc.scalar.dma_start(out=w12c, in_=w12_d[:])
            ones = const_pool.tile([DIM, 1], bf16)
            nc.vector.memset(ones, 1.0)

            for bat in range(NB):
                x1t = xt_pool.tile([P, HEAD, BATCH], bf16, tag="x1t")
                x2t = xt_pool.tile([P, HEAD, BATCH], bf16, tag="x2t")
                if bat == 0:
                    # small first chunk so compute starts asap
                    nc.sync.dma_start(out=x1t[:, 0:2, :],
                                      in_=x1t_d[bat, :, 0:2, :])
                    nc.scalar.dma_start(out=x2t[:, 0:2, :],
                                        in_=x2t_d[bat, :, 0:2, :])
                    nc.sync.dma_start(out=x1t[:, 2:8, :],
                                      in_=x1t_d[bat, :, 2:8, :])
                    nc.scalar.dma_start(out=x2t[:, 2:8, :],
                                        in_=x2t_d[bat, :, 2:8, :])
                else:
                    nc.sync.dma_start(out=x1t, in_=x1t_d[bat, :, :, :])
                    nc.scalar.dma_start(out=x2t, in_=x2t_d[bat, :, :, :])

                rps = rps_pool.tile([1, BATCH], f32)
                p1s = []
                for h in range(HEAD):
                    yps = yps_pool.tile([DIM, BATCH], f32)
                    nc.tensor.matmul(yps, w3l[:, h, :], x1t[:, h, :],
                                     start=True, stop=True)
                    ysb = ysb_pool.tile([DIM, BATCH], bf16)
                    nc.scalar.copy(out=ysb, in_=yps)
                    prod = prod_pool.tile([DIM, BATCH], bf16, tag="prod")
                    nc.vector.tensor_mul(prod, ysb, x2t[:, h, :])
                    # fold t1 into the reduction operand on DVE:
                    # p1 = x1t_h * W1_h[:, None] + prod
                    p1 = p1_pool.tile([DIM, BATCH], bf16, tag="p1")
                    nc.vector.scalar_tensor_tensor(
                        out=p1, in0=x1t[:, h, :], scalar=w12c[:, 0, h:h + 1],
                        in1=prod, op0=mybir.AluOpType.mult,
                        op1=mybir.AluOpType.add,
                    )
                    p1s.append(p1)
                    # t2 accumulates early; doesn't depend on DVE results
                    nc.tensor.matmul(rps, w12c[:, 1, h:h + 1], x2t[:, h, :],
                                     start=(h == 0), stop=False)
                for h in range(HEAD):
                    nc.tensor.matmul(rps, ones, p1s[h],
                                     start=False, stop=(h == HEAD - 1))

                rsb = res_pool.tile([1, BATCH], f32)
                nc.vector.tensor_copy(rsb, rps)
                nc.sync.dma_start(out=out_d[bat, :], in_=rsb)

    nc.finalize()
    return nc


def _prep_weights(W1, W2, W3):
    # W3 is [h, o, i]; lhsT needs [i (partitions), h, o]
    w3t = np.ascontiguousarray(
        np.transpose(np.asarray(W3), (2, 0, 1))).astype(BF16)
    w12 = np.empty((DIM, 2, HEAD), dtype=BF16)
    w12[:, 0, :] = np.asarray(W1).T.astype(BF16)   # [i, h]
    w12[:, 1, :] = np.asarray(W2).T.astype(BF16)   # [o, h]
    return w3t, w12


def _prep_x(x):
    """[B, D] fp32 -> per-core [NB, P, HEAD, BATCH] bf16, pre-transposed."""
    xb = np.asarray(x, dtype=np.float32).astype(BF16)
    # [core, bat, b, h, i] -> [core, bat, i, h, b]
    v = xb.reshape(NCORES, NB, BATCH, HEAD, DIM).transpose(0, 1, 4, 3, 2)
    return np.ascontiguousarray(v)


def kernel(x1, x2, W1, b1, W2, W3):
    if not _nc_cache:
        _nc_cache.append(build_nc())
    nc = _nc_cache[0]

    w3t, w12 = _prep_weights(W1, W2, W3)
    c_b1 = float(np.asarray(b1, dtype=np.float64).sum())
    x1t = _prep_x(x1)
    x2t = _prep_x(x2)

    in_maps = [
        {"x1t": x1t[c], "x2t": x2t[c], "w3t": w3t, "w12": w12}
        for c in range(NCORES)
    ]

    res = run_bass_kernel_spmd(nc, in_maps, core_ids=list(range(NCORES)))
    out = np.concatenate(
        [res.results[c]["out"].reshape(-1) for c in range(NCORES)])
    return (out + np.float32(c_b1)).astype(np.float32)


# revision 17
# speedup vs baseline: 1.0153x; 1.0153x over previous
"""Trainium2 Bass kernel for nn_BilinearFusion.

out[b] = sum_h [ x1_h(b)·W1_h + b1_h + x2_h(b)·W2_h + x2_h(b)^T W3_h x1_h(b) ]

Host-side staging: shard batch across 8 cores; cast x1/x2 to bf16 and lay
them out pre-transposed per head (xt[i, h, b] = x[b, h*128+i]) in batch-major
1MB chunks so the device only does contiguous DMA loads.

Device (per core, 2048 rows = 4 batches x 512):
  loop h over heads (W3_h^T stays stationary across batches):
    per batch:
      Yt[o, b]  = W3_h^T @ xt_h                  (PE, bf16 -> fp32 PSUM)
      prod[o,b] = Yt * x2t_h                      (DVE tensor_mul)
      res_b[1,b] += W1_h^T @ xt_h                 (PE M=1: t1)
      res_b[1,b] += W2_h^T @ x2t_h                (PE M=1: t2)
      res_b[1,b] += ones^T @ prod                 (PE M=1: t3)
  copy each res row to SBUF, DMA out.  Host adds sum(b1).
"""

import numpy as np
import ml_dtypes

import concourse.bass as bass
import concourse.tile as tile
from concourse import bacc, mybir
from concourse.bass_utils import run_bass_kernel_spmd

BF16 = ml_dtypes.bfloat16

B, D, HEAD, DIM = 16384, 1024, 8, 128
NCORES = 8
ROWS = B // NCORES          # 2048 rows per core
P = 128
BATCH = 512                 # rows per batch (moving free dim of matmuls)
NB = ROWS // BATCH          # 4 batches

_nc_cache = []


def build_nc():
    nc = bacc.Bacc(target_bir_lowering=False)
    f32 = mybir.dt.float32
    bf16 = mybir.dt.bfloat16

    x1t_d = nc.dram_tensor("x1t", [NB, P, HEAD, BATCH], bf16,
                           kind="ExternalInput")
    x2t_d = nc.dram_tensor("x2t", [NB, P, HEAD, BATCH], bf16,
                           kind="ExternalInput")
    w3t_d = nc.dram_tensor("w3t", [DIM, HEAD, DIM], bf16, kind="ExternalInput")
    w12_d = nc.dram_tensor("w12", [DIM, 2, HEAD], bf16, kind="ExternalInput")
    out_d = nc.dram_tensor("out", [NB, BATCH], f32, kind="ExternalOutput")

    with tile.TileContext(nc) as tc:
        with (
            tc.tile_pool(name="const", bufs=1) as const_pool,
            tc.tile_pool(name="xt", bufs=3) as xt_pool,
            tc.tile_pool(name="ysb", bufs=4) as ysb_pool,
            tc.tile_pool(name="prod", bufs=4) as prod_pool,
            tc.tile_pool(name="p1", bufs=10) as p1_pool,
            tc.tile_pool(name="res", bufs=2) as res_pool,
            tc.tile_pool(name="yps", bufs=4, space="PSUM") as yps_pool,
            tc.tile_pool(name="rps", bufs=2, space="PSUM") as rps_pool,
        ):
            w3l = const_pool.tile([DIM, HEAD, DIM], bf16)
            nc.sync.dma_start(out=w3l[:, 0:2, :], in_=w3t_d[:, 0:2, :])
            nc.sync.dma_start(out=w3l[:, 2:8, :], in_=w3t_d[:, 2:8, :])
            w12c = const_pool.tile([DIM, 2, HEAD], bf16)
            nc.scalar.dma_start(out=w12c, in_=w12_d[:])
            ones = const_pool.tile([DIM, 1], bf16)
            nc.vector.memset(ones, 1.0)

            for bat in range(NB):
                x1t = xt_pool.tile([P, HEAD, BATCH], bf16, tag="x1t")
                x2t = xt_pool.tile([P, HEAD, BATCH], bf16, tag="x2t")
                if bat == 0:
                    # small first chunk so compute starts asap
                    nc.sync.dma_start(out=x1t[:, 0:2, :],
                                      in_=x1t_d[bat, :, 0:2, :])
                    nc.scalar.dma_start(out=x2t[:, 0:2, :],
                                        in_=x2t_d[bat, :, 0:2, :])
                    nc.sync.dma_start(out=x1t[:, 2:8, :],
                                      in_=x1t_d[bat, :, 2:8, :])
                    nc.scalar.dma_start(out=x2t[:, 2:8, :],
                                        in_=x2t_d[bat, :, 2:8, :])
                else:
                    nc.sync.dma_start(out=x1t, in_=x1t_d[bat, :, :, :])
                    nc.scalar.dma_start(out=x2t, in_=x2t_d[bat, :, :, :])

                rps = rps_pool.tile([1, BATCH], f32)
                prods = []
                for h in range(HEAD):
                    yps = yps_pool.tile([DIM, BATCH], f32)
                    nc.tensor.matmul(yps, w3l[:, h, :], x1t[:, h, :],
                                     start=True, stop=True)
                    prod = p1_pool.tile([DIM, BATCH], bf16, tag="p1")
                    nc.vector.tensor_mul(prod, yps, x2t[:, h, :])
                    prods.append(prod)
                    # t1/t2 accumulate early; don't depend on DVE results
                    nc.tensor.matmul(rps, w12c[:, 0, h:h + 1], x1t[:, h, :],
                                     start=(h == 0), stop=False)
                    nc.tensor.matmul(rps, w12c[:, 1, h:h + 1], x2t[:, h, :],
                                     start=False, stop=False)
                for h in range(HEAD):
                    nc.tensor.matmul(rps, ones, prods[h],
                                     start=False, stop=(h == HEAD - 1))

                rsb = res_pool.tile([1, BATCH], f32)
                nc.vector.tensor_copy(rsb, rps)
                nc.sync.dma_start(out=out_d[bat, :], in_=rsb)

    nc.finalize()
    return nc


def _prep_weights(W1, W2, W3):
    # W3 is [h, o, i]; lhsT needs [i (partitions), h, o]
    w3t = np.ascontiguousarray(
        np.transpose(np.asarray(W3), (2, 0, 1))).astype(BF16)
    w12 = np.empty((DIM, 2, HEAD), dtype=BF16)
    w12[:, 0, :] = np.asarray(W1).T.astype(BF16)   # [i, h]
    w12[:, 1, :] = np.asarray(W2).T.astype(BF16)   # [o, h]
    return w3t, w12


def _prep_x(x):
    """[B, D] fp32 -> per-core [NB, P, HEAD, BATCH] bf16, pre-transposed."""
    xb = np.asarray(x, dtype=np.float32).astype(BF16)
    # [core, bat, b, h, i] -> [core, bat, i, h, b]
    v = xb.reshape(NCORES, NB, BATCH, HEAD, DIM).transpose(0, 1, 4, 3, 2)
    return np.ascontiguousarray(v)


def kernel(x1, x2, W1, b1, W2, W3):
    if not _nc_cache:
        _nc_cache.append(build_nc())
    nc = _nc_cache[0]

    w3t, w12 = _prep_weights(W1, W2, W3)
    c_b1 = float(np.asarray(b1, dtype=np.float64).sum())
    x1t = _prep_x(x1)
    x2t = _prep_x(x2)

    in_maps = [
        {"x1t": x1t[c], "x2t": x2t[c], "w3t": w3t, "w12": w12}
        for c in range(NCORES)
    ]

    res = run_bass_kernel_spmd(nc, in_maps, core_ids=list(range(NCORES)))
    out = np.concatenate(
        [res.results[c]["out"].reshape(-1) for c in range(NCORES)])
    return (out + np.float32(c_b1)).astype(np.float32)


# revision 20
# speedup vs baseline: 1.0749x; 1.0586x over previous
"""Trainium2 Bass kernel for nn_BilinearFusion.

out[b] = sum_h [ x1_h(b)·W1_h + b1_h + x2_h(b)·W2_h + x2_h(b)^T W3_h x1_h(b) ]

Host-side staging: shard batch across 8 cores; cast x1/x2 to bf16 and lay
them out pre-transposed per head (xt[i, h, b] = x[b, h*128+i]) in batch-major
1MB chunks so the device only does contiguous DMA loads.

Device (per core, 2048 rows = 4 batches x 512):
  loop h over heads (W3_h^T stays stationary across batches):
    per batch:
      Yt[o, b]  = W3_h^T @ xt_h                  (PE, bf16 -> fp32 PSUM)
      prod[o,b] = Yt * x2t_h                      (DVE tensor_mul)
      res_b[1,b] += W1_h^T @ xt_h                 (PE M=1: t1)
      res_b[1,b] += W2_h^T @ x2t_h                (PE M=1: t2)
      res_b[1,b] += ones^T @ prod                 (PE M=1: t3)
  copy each res row to SBUF, DMA out.  Host adds sum(b1).
"""

import numpy as np
import ml_dtypes

import concourse.bass as bass
import concourse.tile as tile
from concourse import bacc, mybir
from concourse.bass_utils import run_bass_kernel_spmd

BF16 = ml_dtypes.bfloat16

B, D, HEAD, DIM = 16384, 1024, 8, 128
NCORES = 8
ROWS = B // NCORES          # 2048 rows per core
P = 128
BATCH = 512                 # rows per batch (moving free dim of matmuls)
NB = ROWS // BATCH          # 4 batches

_nc_cache = []


def build_nc():
    nc = bacc.Bacc(target_bir_lowering=False)
    f32 = mybir.dt.float32
    bf16 = mybir.dt.bfloat16

    x1t_d = nc.dram_tensor("x1t", [NB, P, HEAD, BATCH], bf16,
                           kind="ExternalInput")
    x2t_d = nc.dram_tensor("x2t", [NB, P, HEAD, BATCH], bf16,
                           kind="ExternalInput")
    w3t_d = nc.dram_tensor("w3t", [DIM, HEAD, DIM], bf16, kind="ExternalInput")
    w12_d = nc.dram_tensor("w12", [DIM, 2, HEAD], bf16, kind="ExternalInput")
    out_d = nc.dram_tensor("out", [NB, BATCH], f32, kind="ExternalOutput")

    with tile.TileContext(nc) as tc:
        with (
            tc.tile_pool(name="const", bufs=1) as const_pool,
            tc.tile_pool(name="xt", bufs=3) as xt_pool,
            tc.tile_pool(name="ysb", bufs=4) as ysb_pool,
            tc.tile_pool(name="prod", bufs=4) as prod_pool,
            tc.tile_pool(name="p1", bufs=10) as p1_pool,
            tc.tile_pool(name="res", bufs=2) as res_pool,
            tc.tile_pool(name="yps", bufs=6, space="PSUM") as yps_pool,
            tc.tile_pool(name="rps", bufs=2, space="PSUM") as rps_pool,
        ):
            w3l = const_pool.tile([DIM, HEAD, DIM], bf16)
            nc.sync.dma_start(out=w3l[:, 0:2, :], in_=w3t_d[:, 0:2, :])
            # bulky remainder on the idle SWDGE ring so it doesn't delay
            # the first data chunks on the HWDGE rings
            nc.gpsimd.dma_start(out=w3l[:, 2:8, :], in_=w3t_d[:, 2:8, :])
            w12c = const_pool.tile([DIM, 2, HEAD], bf16)
            nc.scalar.dma_start(out=w12c, in_=w12_d[:])
            ones = const_pool.tile([DIM, 1], bf16)
            nc.vector.memset(ones, 1.0)

            for bat in range(NB):
                x1t = xt_pool.tile([P, HEAD, BATCH], bf16, tag="x1t")
                x2t = xt_pool.tile([P, HEAD, BATCH], bf16, tag="x2t")
                if bat == 0:
                    # small chunks so compute starts asap
                    for ck in range(4):
                        a, b = 2 * ck, 2 * ck + 2
                        nc.sync.dma_start(out=x1t[:, a:b, :],
                                          in_=x1t_d[bat, :, a:b, :])
                        nc.scalar.dma_start(out=x2t[:, a:b, :],
                                            in_=x2t_d[bat, :, a:b, :])
                else:
                    nc.sync.dma_start(out=x1t, in_=x1t_d[bat, :, :, :])
                    nc.scalar.dma_start(out=x2t, in_=x2t_d[bat, :, :, :])

                rps = rps_pool.tile([1, BATCH], f32)
                prods = []
                for h in range(HEAD):
                    yps = yps_pool.tile([DIM, BATCH], f32)
                    nc.tensor.matmul(yps, w3l[:, h, :], x1t[:, h, :],
                                     start=True, stop=True)
                    prod = p1_pool.tile([DIM, BATCH], bf16, tag="p1")
                    nc.vector.tensor_mul(prod, yps, x2t[:, h, :])
                    prods.append(prod)
                    # t1/t2 accumulate early; don't depend on DVE results
                    nc.tensor.matmul(rps, w12c[:, 0, h:h + 1], x1t[:, h, :],
                                     start=(h == 0), stop=False)
                    nc.tensor.matmul(rps, w12c[:, 1, h:h + 1], x2t[:, h, :],
                                     start=False, stop=False)
                for h in range(HEAD):
                    nc.tensor.matmul(rps, ones, prods[h],
                                     start=False, stop=(h == HEAD - 1))

                rsb = res_pool.tile([1, BATCH], f32)
                nc.scalar.copy(out=rsb, in_=rps)
                nc.sync.dma_start(out=out_d[bat, :], in_=rsb)

    nc.finalize()
    return nc


def _prep_weights(W1, W2, W3):
    # W3 is [h, o, i]; lhsT needs [i (partitions), h, o]
    w3t = np.ascontiguousarray(
        np.transpose(np.asarray(W3), (2, 0, 1))).astype(BF16)
    w12 = np.empty((DIM, 2, HEAD), dtype=BF16)
    w12[:, 0, :] = np.asarray(W1).T.astype(BF16)   # [i, h]
    w12[:, 1, :] = np.asarray(W2).T.astype(BF16)   # [o, h]
    return w3t, w12


def _prep_x(x):
    """[B, D] fp32 -> per-core [NB, P, HEAD, BATCH] bf16, pre-transposed."""
    xb = np.asarray(x, dtype=np.float32).astype(BF16)
    # [core, bat, b, h, i] -> [core, bat, i, h, b]
    v = xb.reshape(NCORES, NB, BATCH, HEAD, DIM).transpose(0, 1, 4, 3, 2)
    return np.ascontiguousarray(v)


def kernel(x1, x2, W1, b1, W2, W3):
    if not _nc_cache:
        _nc_cache.append(build_nc())
    nc = _nc_cache[0]

    w3t, w12 = _prep_weights(W1, W2, W3)
    c_b1 = float(np.asarray(b1, dtype=np.float64).sum())
    x1t = _prep_x(x1)
    x2t = _prep_x(x2)

    in_maps = [
        {"x1t": x1t[c], "x2t": x2t[c], "w3t": w3t, "w12": w12}
        for c in range(NCORES)
    ]

    res = run_bass_kernel_spmd(nc, in_maps, core_ids=list(range(NCORES)))
    out = np.concatenate(
        [res.results[c]["out"].reshape(-1) for c in range(NCORES)])
    return (out + np.float32(c_b1)).astype(np.float32)
